# revision 15
# baseline (speedup 1.0000x reference)
"""FCOS box decode + class-aware NMS (greedy, IoU>0.5) on 8 Trainium2 cores.

Strategy
--------
The reference does: decode boxes -> class-offset trick -> sort by score ->
dense 8192x8192 pairwise IoU -> sequential greedy suppression sweep.

Key observation: the IoU>0.5 "suppression" predicate is symmetric and boxes
are small (<=256px) relative to the offset coordinate space, so if boxes are
sorted by their (class-offset) left edge x1, a box can only overlap a short
contiguous run of the boxes that follow it: every pair outside the run has
x1[j] > x2[i], which makes the reference's clip(xx2-xx1,0) exactly 0 and
hence IoU exactly 0 (a comparison of stored f32 values - no rounding
assumptions).  The device therefore computes only banded predicate tiles
(64 chunks x [128 rows, W window cols], W from the data, ~256) and ships the
0/1 predicate back as bytes.  The suppression graph is extremely sparse
(~1.7k edges out of 33M candidate pairs), so the greedy sweep itself is
O(edges) host glue, as is the argsort.

Device-side structure per core (8 row-chunks, sorted-x1 order):
 - column coords x1,y1,x2,y2 replicated across partitions (host-prepared,
   DMA split over the three DGE queues, small loads first).
 - a registered custom DVE op computes a whole overlap extent in ONE
   vector pass per chunk and axis: relu(min(x2c,s_x2r) - max(x1c,s_x1r)).
   min/max are exact, the single subtract rounds identically to the
   reference, relu is exact.
 - TensorE computes asum = area_r + area_c via a K=2 ones matmul into
   PSUM (products *1.0; one f32 add; within ~2ulp of IEEE due to PE
   internals).
 - inter = wr*hr (split across Vector and GpSimd), then the fused
   predicate (inter*3) > asum on Vector writing uint8 directly.

Numerical safety: the dataset's minimum distance of any overlapping pair
from the IoU=0.5 threshold is ~157 ulps (measured), while every deviation
this kernel makes from the reference's exact rounding sequence (divide
avoidance, 3*inter form, PE accumulation) is <= ~6 ulps.
"""

import numpy as np

N = 8192
CH = 128                 # rows per chunk (= SBUF partitions)
NCHUNK = N // CH         # 64
NCORES = 8
CPC = NCHUNK // NCORES   # chunks per core
NHALF = 4                # instruction fusion granularity for inter/pred
HPC = CPC // NHALF       # chunks per half

_PROGRAM_CACHE = {}
_EDGE_OP = None


def _get_edge_op():
    """Register (once) the fused overlap-extent custom DVE op:
    out = relu(min(Src0, s0) - max(Src1, s1))."""
    global _EDGE_OP
    if _EDGE_OP is not None:
        return _EDGE_OP
    from concourse import dve_ops
    from concourse.dve_spec import Spec, Src0, Src1, C0, C1, relu, minn, maxx, lower
    from concourse.dve_table_gen import dve_ver_for
    from concourse.dve_uop import DveOpSpec

    name = "NMS_EXTENT_ANT"
    spec = Spec(
        body=relu(minn(Src0, C0) - maxx(Src1, C1)),
        reference=lambda in0, in1, s0, s1, imm2=0.0: np.maximum(
            np.minimum(in0, s0) - np.maximum(in1, s1), 0
        ),
    )
    if name not in dve_ops._SUB_OPCODE_FOR_NAME:
        # self-consistent sha pin (computed from this build's lower())
        shas = {}
        for ver in ("v3", "v4"):
            try:
                tmp = DveOpSpec(name=name, uops=lower(spec, ver=ver))
                shas[ver] = tmp.sha(ver)
            except Exception:
                pass
        op = dve_ops.DveOp(name, spec, subdim=False, uops_sha=shas)
        dve_ops.OPS.append(op)
        dve_ops._SUB_OPCODE_FOR_NAME[name] = (
            dve_ops._CUSTOM_DVE_ROW_BASE + len(dve_ops.OPS) - 1
        )
        dve_ops.CUSTOM_DVE_SPECS[name] = spec
        assert max(dve_ops._SUB_OPCODE_FOR_NAME.values()) < 0x20
        _EDGE_OP = op
    else:
        _EDGE_OP = next(o for o in dve_ops.OPS if o.name == name)
    return _EDGE_OP


def _build_program(W, stride_f):
    import concourse.bass as bass
    import concourse.tile as tile
    from concourse import bacc, mybir

    edge_op = _get_edge_op()

    f32 = mybir.dt.float32
    u8 = mybir.dt.uint8
    Alu = mybir.AluOpType
    Lc = CPC * CH + W    # columns a core needs (its 8 chunks + trailing window)
    LA = HPC * CH + W    # columns needed by the first half (loaded first)

    nc = bacc.Bacc(
        "TRN2", target_bir_lowering=False, debug=False, num_devices=NCORES
    )

    deltas_t = nc.dram_tensor("deltas", [N, 4], f32, kind="ExternalInput").ap()
    locs_t = nc.dram_tensor("locations", [N, 2], f32, kind="ExternalInput").ap()
    # host-replicated column coords x1,y1,x2,y2 (window slice)
    cols_t = nc.dram_tensor("cols", [128, 4, Lc], f32, kind="ExternalInput").ap()
    # per-core row-box coords: [128, CPC*5] (x1,y1,x2,y2,area per chunk)
    rows_t = nc.dram_tensor("rows", [128, CPC * 5], f32, kind="ExternalInput").ap()
    # asum matmul operands: lhsT rows = [area_r ; 1], rhs rows = [1 ; area_c]
    mml_t = nc.dram_tensor("mml", [2, CPC * CH], f32, kind="ExternalInput").ap()
    mmr_t = nc.dram_tensor("mmr", [2, Lc], f32, kind="ExternalInput").ap()
    boxes_o = nc.dram_tensor("boxes", [N, 4], f32, kind="ExternalOutput").ap()
    pred_o = nc.dram_tensor(
        "pred", [CPC, 128, W], u8, kind="ExternalOutput"
    ).ap()

    AP = bass.AP

    with tile.TileContext(nc) as tc:
        with (
            tc.tile_pool(name="const", bufs=1) as cp,
            tc.tile_pool(name="work", bufs=3) as wp,
            tc.tile_pool(name="psum", bufs=3, space="PSUM") as pp,
        ):
            # ---- small loads first (cheap; unblock decode + PE early) ----
            dl = wp.tile([128, 64, 4], f32, tag="dl")
            nc.sync.dma_start(dl[:], deltas_t.rearrange("(p f) c -> p f c", p=128))
            ll = wp.tile([128, 64, 2], f32, tag="ll")
            nc.scalar.dma_start(ll[:], locs_t.rearrange("(p f) c -> p f c", p=128))
            rowsb = cp.tile([128, CPC * 5], f32, tag="rowsb")
            nc.gpsimd.dma_start(rowsb[:], rows_t[:])
            rt = rowsb[:].tensor
            mmlb = cp.tile([2, CPC * CH], f32, tag="mmlb")
            nc.scalar.dma_start(mmlb[:], mml_t[:])
            mmrb = cp.tile([2, Lc], f32, tag="mmrb")
            nc.scalar.dma_start(mmrb[:], mmr_t[:])

            # ---- replicated cols: first-quarter window first, across queues ----
            colsb = cp.tile([128, 4, Lc], f32, tag="colsb")
            # piece A (first NHALF=... quarter windows) split across all three
            # DGE queues so the first compute isn't waiting on one queue
            third = (LA + 2) // 3
            nc.sync.dma_start(colsb[:, :, 0:third], cols_t[:, :, 0:third])
            nc.scalar.dma_start(
                colsb[:, :, third : 2 * third], cols_t[:, :, third : 2 * third]
            )
            nc.gpsimd.dma_start(
                colsb[:, :, 2 * third : LA], cols_t[:, :, 2 * third : LA]
            )
            nc.sync.dma_start(colsb[:, :, LA:Lc], cols_t[:, :, LA:Lc])
            ct = colsb[:].tensor

            # ---- box decode (original order), all cores redundantly ----
            dr = wp.tile([128, 64, 4], f32, tag="dr")
            nc.vector.tensor_scalar_max(dr[:], dl[:], 0.0)
            bx = wp.tile([128, 64, 4], f32, tag="bx")
            for c, (sgn, lc) in enumerate([(-1, 0), (-1, 1), (1, 0), (1, 1)]):
                nc.vector.scalar_tensor_tensor(
                    bx[:, :, c], dr[:, :, c], sgn * stride_f, ll[:, :, lc],
                    op0=Alu.mult, op1=Alu.add,
                )
            nc.sync.dma_start(boxes_o.rearrange("(p f) c -> p f c", p=128), bx[:])

            def cwin1(q, t):  # single chunk t (global in 0..CPC): [128, W]
                return AP(ct, q * Lc + t * CH, [[4 * Lc, 128], [1, W]])

            def rcol(q, t):  # [128,1] scalar AP for chunk t
                return rowsb[:, t * 5 + q : t * 5 + q + 1]

            for h in range(NHALF):
                # asum on TensorE: psum[p, f] = area_r[p]*1 + 1*area_c[f]
                asum = pp.tile([128, HPC, W], f32, tag="asum")
                for tl in range(HPC):
                    t = h * HPC + tl
                    nc.tensor.matmul(
                        asum[:, tl, :],
                        mmlb[:, t * CH : (t + 1) * CH],
                        mmrb[:, t * CH : t * CH + W],
                        start=True, stop=True,
                    )

                # fused overlap extents: one custom-DVE pass per chunk+axis
                wr = wp.tile([128, HPC, W], f32, tag="wr")
                hr = wp.tile([128, HPC, W], f32, tag="hr")
                for tl in range(HPC):
                    t = h * HPC + tl
                    nc.vector._custom_dve(
                        edge_op, out=wr[:, tl, :],
                        in0=cwin1(2, t), in1=cwin1(0, t),
                        s0=rcol(2, t), s1=rcol(0, t),
                    )
                    nc.vector._custom_dve(
                        edge_op, out=hr[:, tl, :],
                        in0=cwin1(3, t), in1=cwin1(1, t),
                        s0=rcol(3, t), s1=rcol(1, t),
                    )

                inter = wp.tile([128, HPC, W], f32, tag="inter")
                nc.vector.tensor_tensor(inter[:], wr[:], hr[:], op=Alu.mult)

                # pred = (inter*3) > asum   (safe: >=150 ulp data margin)
                pr = wp.tile([128, HPC, W], u8, tag="pr")
                nc.vector.scalar_tensor_tensor(
                    pr[:], inter[:], 3.0, asum[:], op0=Alu.mult, op1=Alu.is_gt
                )
                # one DMA per quarter; dram AP reordered to match [128, HPC, W]
                (nc.sync if h % 2 == 0 else nc.scalar).dma_start(
                    AP(pred_o.tensor, h * HPC * 128 * W,
                       [[W, 128], [128 * W, HPC], [1, W]]),
                    pr[:],
                )

    nc.compile()
    return nc


def kernel(deltas, locations, scores, class_ids, stride):
    deltas = np.asarray(deltas, np.float32)
    locations = np.asarray(locations, np.float32)
    scores = np.asarray(scores, np.float32)
    class_ids = np.asarray(class_ids, np.int32)
    stride_f = float(np.asarray(stride))
    n = deltas.shape[0]
    assert n == N

    # ---- host: decode (for sort/band prep only; boxes output comes from
    # the device), class offsets, x1-sort, window size ----
    dd = np.clip(deltas, 0, None)
    xc, yc = locations[:, 0], locations[:, 1]
    s8 = np.float32(stride_f)
    bx = np.stack(
        [xc - dd[:, 0] * s8, yc - dd[:, 1] * s8,
         xc + dd[:, 2] * s8, yc + dd[:, 3] * s8], axis=1
    ).astype(np.float32)
    mc = bx.max()
    off = (class_ids.astype(np.float32) * (mc + np.float32(1.0))).astype(np.float32)
    b = (bx + off[:, None]).astype(np.float32)
    areas = ((b[:, 2] - b[:, 0]) * (b[:, 3] - b[:, 1])).astype(np.float32)

    xorder = np.argsort(b[:, 0], kind="stable")
    bsx = b[xorder]
    areax = areas[xorder]
    x1s, y1s, x2s, y2s = bsx[:, 0], bsx[:, 1], bsx[:, 2], bsx[:, 3]

    # exact candidate window: for row i all j>i with x1[j] <= x2[i]
    ends = np.searchsorted(x1s, x2s, side="right")
    wneed = max(
        int(ends[t * CH : (t + 1) * CH].max()) - t * CH for t in range(NCHUNK)
    )
    W = max(256, int(np.ceil(wneed / 128.0)) * 128)

    key = (W, stride_f)
    if key not in _PROGRAM_CACHE:
        _PROGRAM_CACHE[key] = _build_program(W, stride_f)
    nc = _PROGRAM_CACHE[key]

    # ---- padded column arrays (pad boxes can never overlap: x1=+huge) ----
    Lc = CPC * CH + W
    PAD = np.float32(3e38)
    L = n + W
    colq = np.empty((5, L), np.float32)
    colq[0, :n] = x1s; colq[0, n:] = PAD
    colq[1, :n] = y1s; colq[1, n:] = PAD
    colq[2, :n] = x2s; colq[2, n:] = PAD
    colq[3, :n] = y2s; colq[3, n:] = PAD
    colq[4, :n] = areax; colq[4, n:] = 0.0

    rowq = np.stack([x1s, y1s, x2s, y2s, areax], axis=1)  # [n, 5]

    in_maps = []
    for c in range(NCORES):
        s0 = c * CPC * CH
        cols_c = np.ascontiguousarray(
            np.broadcast_to(colq[None, :4, s0 : s0 + Lc], (128, 4, Lc))
        )
        rows_c = np.ascontiguousarray(
            rowq[s0 : s0 + CPC * CH].reshape(CPC, CH, 5).transpose(1, 0, 2)
        ).reshape(128, CPC * 5)
        mml_c = np.ones((2, CPC * CH), np.float32)
        mml_c[0] = areax[s0 : s0 + CPC * CH]
        mmr_c = np.ones((2, Lc), np.float32)
        mmr_c[1] = colq[4, s0 : s0 + Lc]
        in_maps.append(
            {
                "deltas": deltas,
                "locations": locations,
                "cols": cols_c,
                "rows": rows_c,
                "mml": mml_c,
                "mmr": mmr_c,
            }
        )

    from concourse import bass_utils

    res = bass_utils.run_bass_kernel_spmd(
        nc, in_maps, core_ids=list(range(NCORES))
    )
    results = res.results

    boxes_out = results[0]["boxes"]

    predb = np.concatenate(
        [results[c]["pred"] for c in range(NCORES)], axis=0
    )  # [NCHUNK, 128, W]

    # ---- host: edge extraction + greedy sweep ----
    tt, pp, ff = np.nonzero(predb)
    i = tt * CH + pp
    j = tt * CH + ff
    m = (j > i) & (j < n)
    i, j = i[m], j[m]

    sorder = np.argsort(-scores, kind="stable")
    srank = np.empty(n, np.int64)
    srank[sorder] = np.arange(n)
    si = srank[xorder[i]]
    sj = srank[xorder[j]]
    lo = np.minimum(si, sj)
    hi = np.maximum(si, sj)

    keep_s = np.ones(n, bool)
    if len(lo):
        perm = np.argsort(lo, kind="stable")
        lo, hi = lo[perm], hi[perm]
        uniq, start = np.unique(lo, return_index=True)
        start = np.append(start, len(lo))
        for k in range(len(uniq)):
            if keep_s[uniq[k]]:
                keep_s[hi[start[k] : start[k + 1]]] = False
    keep_mask = np.zeros(n, bool)
    keep_mask[sorder] = keep_s

    return boxes_out, keep_mask


# revision 16
# speedup vs baseline: 1.0929x; 1.0929x over previous
"""FCOS box decode + class-aware NMS (greedy, IoU>0.5) on 8 Trainium2 cores.

Strategy
--------
The reference does: decode boxes -> class-offset trick -> sort by score ->
dense 8192x8192 pairwise IoU -> sequential greedy suppression sweep.

Key observation: the IoU>0.5 "suppression" predicate is symmetric and boxes
are small (<=256px) relative to the offset coordinate space, so if boxes are
sorted by their (class-offset) left edge x1, a box can only overlap a short
contiguous run of the boxes that follow it: every pair outside the run has
x1[j] > x2[i], which makes the reference's clip(xx2-xx1,0) exactly 0 and
hence IoU exactly 0 (a comparison of stored f32 values - no rounding
assumptions).  The device therefore computes only banded predicate tiles
(64 chunks x [128 rows, W window cols], W from the data, ~256) and ships the
0/1 predicate back as bytes.  The suppression graph is extremely sparse
(~1.7k edges out of 33M candidate pairs), so the greedy sweep itself is
O(edges) host glue, as is the argsort.

Device-side structure per core (8 row-chunks, sorted-x1 order):
 - column coords x1,y1,x2,y2 replicated across partitions (host-prepared,
   DMA split over the three DGE queues, small loads first).
 - a registered custom DVE op computes a whole overlap extent in ONE
   vector pass per chunk and axis: relu(min(x2c,s_x2r) - max(x1c,s_x1r)).
   min/max are exact, the single subtract rounds identically to the
   reference, relu is exact.
 - TensorE computes asum = area_r + area_c via a K=2 ones matmul into
   PSUM (products *1.0; one f32 add; within ~2ulp of IEEE due to PE
   internals).
 - inter = wr*hr (split across Vector and GpSimd), then the fused
   predicate (inter*3) > asum on Vector writing uint8 directly.

Numerical safety: the dataset's minimum distance of any overlapping pair
from the IoU=0.5 threshold is ~157 ulps (measured), while every deviation
this kernel makes from the reference's exact rounding sequence (divide
avoidance, 3*inter form, PE accumulation) is <= ~6 ulps.
"""

import numpy as np

N = 8192
CH = 128                 # rows per chunk (= SBUF partitions)
NCHUNK = N // CH         # 64
NCORES = 8
CPC = NCHUNK // NCORES   # chunks per core
NHALF = 2                # instruction fusion granularity for inter/pred
HPC = CPC // NHALF       # chunks per half

_PROGRAM_CACHE = {}
_EDGE_OP = None


def _get_edge_op():
    """Register (once) the fused overlap-extent custom DVE op:
    out = relu(min(Src0, s0) - max(Src1, s1))."""
    global _EDGE_OP
    if _EDGE_OP is not None:
        return _EDGE_OP
    from concourse import dve_ops
    from concourse.dve_spec import Spec, Src0, Src1, C0, C1, relu, minn, maxx, lower
    from concourse.dve_table_gen import dve_ver_for
    from concourse.dve_uop import DveOpSpec

    name = "NMS_EXTENT_ANT"
    spec = Spec(
        body=relu(minn(Src0, C0) - maxx(Src1, C1)),
        reference=lambda in0, in1, s0, s1, imm2=0.0: np.maximum(
            np.minimum(in0, s0) - np.maximum(in1, s1), 0
        ),
    )
    if name not in dve_ops._SUB_OPCODE_FOR_NAME:
        # self-consistent sha pin (computed from this build's lower())
        shas = {}
        for ver in ("v3", "v4"):
            try:
                tmp = DveOpSpec(name=name, uops=lower(spec, ver=ver))
                shas[ver] = tmp.sha(ver)
            except Exception:
                pass
        op = dve_ops.DveOp(name, spec, subdim=False, uops_sha=shas)
        dve_ops.OPS.append(op)
        dve_ops._SUB_OPCODE_FOR_NAME[name] = (
            dve_ops._CUSTOM_DVE_ROW_BASE + len(dve_ops.OPS) - 1
        )
        dve_ops.CUSTOM_DVE_SPECS[name] = spec
        assert max(dve_ops._SUB_OPCODE_FOR_NAME.values()) < 0x20
        _EDGE_OP = op
    else:
        _EDGE_OP = next(o for o in dve_ops.OPS if o.name == name)
    return _EDGE_OP


def _build_program(W, stride_f):
    import concourse.bass as bass
    import concourse.tile as tile
    from concourse import bacc, mybir

    edge_op = _get_edge_op()

    f32 = mybir.dt.float32
    u8 = mybir.dt.uint8
    Alu = mybir.AluOpType
    Lc = CPC * CH + W    # columns a core needs (its 8 chunks + trailing window)
    LA = HPC * CH + W    # columns needed by the first half (loaded first)

    nc = bacc.Bacc(
        "TRN2", target_bir_lowering=False, debug=False, num_devices=NCORES
    )

    deltas_t = nc.dram_tensor("deltas", [N, 4], f32, kind="ExternalInput").ap()
    locs_t = nc.dram_tensor("locations", [N, 2], f32, kind="ExternalInput").ap()
    # host-replicated column coords x1,y1,x2,y2 (window slice)
    cols_t = nc.dram_tensor("cols", [128, 4, Lc], f32, kind="ExternalInput").ap()
    # per-core row-box coords: [128, CPC*5] (x1,y1,x2,y2,area per chunk)
    rows_t = nc.dram_tensor("rows", [128, CPC * 5], f32, kind="ExternalInput").ap()
    # asum matmul operands: lhsT rows = [area_r ; 1], rhs rows = [1 ; area_c]
    mml_t = nc.dram_tensor("mml", [2, CPC * CH], f32, kind="ExternalInput").ap()
    mmr_t = nc.dram_tensor("mmr", [2, Lc], f32, kind="ExternalInput").ap()
    boxes_o = nc.dram_tensor("boxes", [N, 4], f32, kind="ExternalOutput").ap()
    pred_o = nc.dram_tensor(
        "pred", [CPC, 128, W], u8, kind="ExternalOutput"
    ).ap()

    AP = bass.AP

    with tile.TileContext(nc) as tc:
        with (
            tc.tile_pool(name="const", bufs=1) as cp,
            tc.tile_pool(name="work", bufs=3) as wp,
            tc.tile_pool(name="psum", bufs=3, space="PSUM") as pp,
        ):
            # ---- small loads first (cheap; unblock decode + PE early) ----
            dl = wp.tile([128, 64, 4], f32, tag="dl")
            nc.sync.dma_start(dl[:], deltas_t.rearrange("(p f) c -> p f c", p=128))
            ll = wp.tile([128, 64, 2], f32, tag="ll")
            nc.scalar.dma_start(ll[:], locs_t.rearrange("(p f) c -> p f c", p=128))
            rowsb = cp.tile([128, CPC * 5], f32, tag="rowsb")
            nc.gpsimd.dma_start(rowsb[:], rows_t[:])
            rt = rowsb[:].tensor
            mmlb = cp.tile([2, CPC * CH], f32, tag="mmlb")
            nc.scalar.dma_start(mmlb[:], mml_t[:])
            mmrb = cp.tile([2, Lc], f32, tag="mmrb")
            nc.scalar.dma_start(mmrb[:], mmr_t[:])

            # ---- replicated cols: first-quarter window first, across queues ----
            colsb = cp.tile([128, 4, Lc], f32, tag="colsb")
            # piece A (first NHALF=... quarter windows) split across all three
            # DGE queues so the first compute isn't waiting on one queue
            third = (LA + 2) // 3
            nc.sync.dma_start(colsb[:, :, 0:third], cols_t[:, :, 0:third])
            nc.scalar.dma_start(
                colsb[:, :, third : 2 * third], cols_t[:, :, third : 2 * third]
            )
            nc.gpsimd.dma_start(
                colsb[:, :, 2 * third : LA], cols_t[:, :, 2 * third : LA]
            )
            nc.sync.dma_start(colsb[:, :, LA:Lc], cols_t[:, :, LA:Lc])
            ct = colsb[:].tensor

            # ---- box decode (original order), all cores redundantly ----
            dr = wp.tile([128, 64, 4], f32, tag="dr")
            nc.vector.tensor_scalar_max(dr[:], dl[:], 0.0)
            bx = wp.tile([128, 64, 4], f32, tag="bx")
            for c, (sgn, lc) in enumerate([(-1, 0), (-1, 1), (1, 0), (1, 1)]):
                nc.vector.scalar_tensor_tensor(
                    bx[:, :, c], dr[:, :, c], sgn * stride_f, ll[:, :, lc],
                    op0=Alu.mult, op1=Alu.add,
                )
            nc.sync.dma_start(boxes_o.rearrange("(p f) c -> p f c", p=128), bx[:])

            def cwin1(q, t):  # single chunk t (global in 0..CPC): [128, W]
                return AP(ct, q * Lc + t * CH, [[4 * Lc, 128], [1, W]])

            def rcol(q, t):  # [128,1] scalar AP for chunk t
                return rowsb[:, t * 5 + q : t * 5 + q + 1]

            for h in range(NHALF):
                # asum on TensorE: psum[p, f] = area_r[p]*1 + 1*area_c[f]
                asum = pp.tile([128, HPC, W], f32, tag="asum")
                for tl in range(HPC):
                    t = h * HPC + tl
                    nc.tensor.matmul(
                        asum[:, tl, :],
                        mmlb[:, t * CH : (t + 1) * CH],
                        mmrb[:, t * CH : t * CH + W],
                        start=True, stop=True,
                    )

                # fused overlap extents: one custom-DVE pass per chunk+axis
                wr = wp.tile([128, HPC, W], f32, tag="wr")
                hr = wp.tile([128, HPC, W], f32, tag="hr")
                for tl in range(HPC):
                    t = h * HPC + tl
                    nc.vector._custom_dve(
                        edge_op, out=wr[:, tl, :],
                        in0=cwin1(2, t), in1=cwin1(0, t),
                        s0=rcol(2, t), s1=rcol(0, t),
                    )
                    nc.vector._custom_dve(
                        edge_op, out=hr[:, tl, :],
                        in0=cwin1(3, t), in1=cwin1(1, t),
                        s0=rcol(3, t), s1=rcol(1, t),
                    )

                inter = wp.tile([128, HPC, W], f32, tag="inter")
                nc.vector.tensor_tensor(inter[:], wr[:], hr[:], op=Alu.mult)

                # pred = (inter*3) > asum   (safe: >=150 ulp data margin)
                pr = wp.tile([128, HPC, W], u8, tag="pr")
                nc.vector.scalar_tensor_tensor(
                    pr[:], inter[:], 3.0, asum[:], op0=Alu.mult, op1=Alu.is_gt
                )
                # one DMA per quarter; dram AP reordered to match [128, HPC, W]
                (nc.sync if h % 2 == 0 else nc.scalar).dma_start(
                    AP(pred_o.tensor, h * HPC * 128 * W,
                       [[W, 128], [128 * W, HPC], [1, W]]),
                    pr[:],
                )

    nc.compile()
    return nc


def kernel(deltas, locations, scores, class_ids, stride):
    deltas = np.asarray(deltas, np.float32)
    locations = np.asarray(locations, np.float32)
    scores = np.asarray(scores, np.float32)
    class_ids = np.asarray(class_ids, np.int32)
    stride_f = float(np.asarray(stride))
    n = deltas.shape[0]
    assert n == N

    # ---- host: decode (for sort/band prep only; boxes output comes from
    # the device), class offsets, x1-sort, window size ----
    dd = np.clip(deltas, 0, None)
    xc, yc = locations[:, 0], locations[:, 1]
    s8 = np.float32(stride_f)
    bx = np.stack(
        [xc - dd[:, 0] * s8, yc - dd[:, 1] * s8,
         xc + dd[:, 2] * s8, yc + dd[:, 3] * s8], axis=1
    ).astype(np.float32)
    mc = bx.max()
    off = (class_ids.astype(np.float32) * (mc + np.float32(1.0))).astype(np.float32)
    b = (bx + off[:, None]).astype(np.float32)
    areas = ((b[:, 2] - b[:, 0]) * (b[:, 3] - b[:, 1])).astype(np.float32)

    xorder = np.argsort(b[:, 0], kind="stable")
    bsx = b[xorder]
    areax = areas[xorder]
    x1s, y1s, x2s, y2s = bsx[:, 0], bsx[:, 1], bsx[:, 2], bsx[:, 3]

    # exact candidate window: for row i all j>i with x1[j] <= x2[i]
    ends = np.searchsorted(x1s, x2s, side="right")
    wneed = max(
        int(ends[t * CH : (t + 1) * CH].max()) - t * CH for t in range(NCHUNK)
    )
    W = max(256, int(np.ceil(wneed / 128.0)) * 128)

    key = (W, stride_f)
    if key not in _PROGRAM_CACHE:
        _PROGRAM_CACHE[key] = _build_program(W, stride_f)
    nc = _PROGRAM_CACHE[key]

    # ---- padded column arrays (pad boxes can never overlap: x1=+huge) ----
    Lc = CPC * CH + W
    PAD = np.float32(3e38)
    L = n + W
    colq = np.empty((5, L), np.float32)
    colq[0, :n] = x1s; colq[0, n:] = PAD
    colq[1, :n] = y1s; colq[1, n:] = PAD
    colq[2, :n] = x2s; colq[2, n:] = PAD
    colq[3, :n] = y2s; colq[3, n:] = PAD
    colq[4, :n] = areax; colq[4, n:] = 0.0

    rowq = np.stack([x1s, y1s, x2s, y2s, areax], axis=1)  # [n, 5]

    in_maps = []
    for c in range(NCORES):
        s0 = c * CPC * CH
        cols_c = np.ascontiguousarray(
            np.broadcast_to(colq[None, :4, s0 : s0 + Lc], (128, 4, Lc))
        )
        rows_c = np.ascontiguousarray(
            rowq[s0 : s0 + CPC * CH].reshape(CPC, CH, 5).transpose(1, 0, 2)
        ).reshape(128, CPC * 5)
        mml_c = np.ones((2, CPC * CH), np.float32)
        mml_c[0] = areax[s0 : s0 + CPC * CH]
        mmr_c = np.ones((2, Lc), np.float32)
        mmr_c[1] = colq[4, s0 : s0 + Lc]
        in_maps.append(
            {
                "deltas": deltas,
                "locations": locations,
                "cols": cols_c,
                "rows": rows_c,
                "mml": mml_c,
                "mmr": mmr_c,
            }
        )

    from concourse import bass_utils

    res = bass_utils.run_bass_kernel_spmd(
        nc, in_maps, core_ids=list(range(NCORES))
    )
    results = res.results

    boxes_out = results[0]["boxes"]

    predb = np.concatenate(
        [results[c]["pred"] for c in range(NCORES)], axis=0
    )  # [NCHUNK, 128, W]

    # ---- host: edge extraction + greedy sweep ----
    tt, pp, ff = np.nonzero(predb)
    i = tt * CH + pp
    j = tt * CH + ff
    m = (j > i) & (j < n)
    i, j = i[m], j[m]

    sorder = np.argsort(-scores, kind="stable")
    srank = np.empty(n, np.int64)
    srank[sorder] = np.arange(n)
    si = srank[xorder[i]]
    sj = srank[xorder[j]]
    lo = np.minimum(si, sj)
    hi = np.maximum(si, sj)

    keep_s = np.ones(n, bool)
    if len(lo):
        perm = np.argsort(lo, kind="stable")
        lo, hi = lo[perm], hi[perm]
        uniq, start = np.unique(lo, return_index=True)
        start = np.append(start, len(lo))
        for k in range(len(uniq)):
            if keep_s[uniq[k]]:
                keep_s[hi[start[k] : start[k + 1]]] = False
    keep_mask = np.zeros(n, bool)
    keep_mask[sorder] = keep_s

    return boxes_out, keep_mask


# revision 17
# speedup vs baseline: 1.1499x; 1.0522x over previous
"""FCOS box decode + class-aware NMS (greedy, IoU>0.5) on 8 Trainium2 cores.

Strategy
--------
The reference does: decode boxes -> class-offset trick -> sort by score ->
dense 8192x8192 pairwise IoU -> sequential greedy suppression sweep.

Key observation: the IoU>0.5 "suppression" predicate is symmetric and boxes
are small (<=256px) relative to the offset coordinate space, so if boxes are
sorted by their (class-offset) left edge x1, a box can only overlap a short
contiguous run of the boxes that follow it: every pair outside the run has
x1[j] > x2[i], which makes the reference's clip(xx2-xx1,0) exactly 0 and
hence IoU exactly 0 (a comparison of stored f32 values - no rounding
assumptions).  The device therefore computes only banded predicate tiles
(64 chunks x [128 rows, W window cols], W from the data, ~256) and ships the
0/1 predicate back as bytes.  The suppression graph is extremely sparse
(~1.7k edges out of 33M candidate pairs), so the greedy sweep itself is
O(edges) host glue, as is the argsort.

Device-side structure per core (8 row-chunks, sorted-x1 order):
 - column coords x1,y1,x2,y2 replicated across partitions (host-prepared,
   DMA split over the three DGE queues, small loads first).
 - a registered custom DVE op computes a whole overlap extent in ONE
   vector pass per chunk and axis: relu(min(x2c,s_x2r) - max(x1c,s_x1r)).
   min/max are exact, the single subtract rounds identically to the
   reference, relu is exact.
 - TensorE computes asum = area_r + area_c via a K=2 ones matmul into
   PSUM (products *1.0; one f32 add; within ~2ulp of IEEE due to PE
   internals).
 - inter = wr*hr (split across Vector and GpSimd), then the fused
   predicate (inter*3) > asum on Vector writing uint8 directly.

Numerical safety: the dataset's minimum distance of any overlapping pair
from the IoU=0.5 threshold is ~157 ulps (measured), while every deviation
this kernel makes from the reference's exact rounding sequence (divide
avoidance, 3*inter form, PE accumulation) is <= ~6 ulps.
"""

import numpy as np

N = 8192
CH = 128                 # rows per chunk (= SBUF partitions)
NCHUNK = N // CH         # 64
NCORES = 8
CPC = NCHUNK // NCORES   # chunks per core
NHALF = 2                # instruction fusion granularity for inter/pred
HPC = CPC // NHALF       # chunks per half

_PROGRAM_CACHE = {}
_EDGE_OP = None


def _get_edge_op():
    """Register (once) the fused overlap-extent custom DVE op:
    out = relu(min(Src0, s0) - max(Src1, s1))."""
    global _EDGE_OP
    if _EDGE_OP is not None:
        return _EDGE_OP
    from concourse import dve_ops
    from concourse.dve_spec import Spec, Src0, Src1, C0, C1, relu, minn, maxx, lower
    from concourse.dve_table_gen import dve_ver_for
    from concourse.dve_uop import DveOpSpec

    name = "NMS_EXTENT_ANT"
    spec = Spec(
        body=relu(minn(Src0, C0) - maxx(Src1, C1)),
        reference=lambda in0, in1, s0, s1, imm2=0.0: np.maximum(
            np.minimum(in0, s0) - np.maximum(in1, s1), 0
        ),
    )
    if name not in dve_ops._SUB_OPCODE_FOR_NAME:
        # self-consistent sha pin (computed from this build's lower())
        shas = {}
        for ver in ("v3", "v4"):
            try:
                tmp = DveOpSpec(name=name, uops=lower(spec, ver=ver))
                shas[ver] = tmp.sha(ver)
            except Exception:
                pass
        op = dve_ops.DveOp(name, spec, subdim=False, uops_sha=shas)
        dve_ops.OPS.append(op)
        dve_ops._SUB_OPCODE_FOR_NAME[name] = (
            dve_ops._CUSTOM_DVE_ROW_BASE + len(dve_ops.OPS) - 1
        )
        dve_ops.CUSTOM_DVE_SPECS[name] = spec
        assert max(dve_ops._SUB_OPCODE_FOR_NAME.values()) < 0x20
        _EDGE_OP = op
    else:
        _EDGE_OP = next(o for o in dve_ops.OPS if o.name == name)
    return _EDGE_OP


def _build_program(W, stride_f):
    import concourse.bass as bass
    import concourse.tile as tile
    from concourse import bacc, mybir

    edge_op = _get_edge_op()

    f32 = mybir.dt.float32
    u8 = mybir.dt.uint8
    Alu = mybir.AluOpType
    Lc = CPC * CH + W    # columns a core needs (its 8 chunks + trailing window)
    LA = HPC * CH + W    # columns needed by the first half (loaded first)

    nc = bacc.Bacc(
        "TRN2", target_bir_lowering=False, debug=False, num_devices=NCORES
    )

    deltas_t = nc.dram_tensor("deltas", [N, 4], f32, kind="ExternalInput").ap()
    locs_t = nc.dram_tensor("locations", [N, 2], f32, kind="ExternalInput").ap()
    # host-replicated column coords x1,y1,x2,y2 (window slice)
    cols_t = nc.dram_tensor("cols", [128, 4, Lc], f32, kind="ExternalInput").ap()
    # per-core row-box coords: [128, CPC*5] (x1,y1,x2,y2,area per chunk)
    rows_t = nc.dram_tensor("rows", [128, CPC * 5], f32, kind="ExternalInput").ap()
    # asum matmul operands: lhsT rows = [area_r ; 1], rhs rows = [1 ; area_c]
    mml_t = nc.dram_tensor("mml", [2, CPC * CH], f32, kind="ExternalInput").ap()
    mmr_t = nc.dram_tensor("mmr", [2, Lc], f32, kind="ExternalInput").ap()
    boxes_o = nc.dram_tensor("boxes", [N, 4], f32, kind="ExternalOutput").ap()
    pred_o = nc.dram_tensor(
        "pred", [CPC, 128, W], u8, kind="ExternalOutput"
    ).ap()

    AP = bass.AP

    with tile.TileContext(nc) as tc:
        with (
            tc.tile_pool(name="const", bufs=1) as cp,
            tc.tile_pool(name="work", bufs=3) as wp,
            tc.tile_pool(name="psum", bufs=3, space="PSUM") as pp,
        ):
            # ---- small loads first (cheap; unblock decode + PE early) ----
            dl = wp.tile([128, 64, 4], f32, tag="dl")
            nc.sync.dma_start(dl[:], deltas_t.rearrange("(p f) c -> p f c", p=128))
            ll = wp.tile([128, 64, 2], f32, tag="ll")
            nc.scalar.dma_start(ll[:], locs_t.rearrange("(p f) c -> p f c", p=128))
            rowsb = cp.tile([128, CPC * 5], f32, tag="rowsb")
            nc.gpsimd.dma_start(rowsb[:], rows_t[:])
            rt = rowsb[:].tensor
            mmlb = cp.tile([2, CPC * CH], f32, tag="mmlb")
            nc.scalar.dma_start(mmlb[:], mml_t[:])
            mmrb = cp.tile([2, Lc], f32, tag="mmrb")
            nc.scalar.dma_start(mmrb[:], mmr_t[:])

            # ---- replicated cols: first-quarter window first, across queues ----
            colsb = cp.tile([128, 4, Lc], f32, tag="colsb")
            # piece A (first NHALF=... quarter windows) split across all three
            # DGE queues so the first compute isn't waiting on one queue
            third = (LA + 2) // 3
            nc.sync.dma_start(colsb[:, :, 0:third], cols_t[:, :, 0:third])
            nc.scalar.dma_start(
                colsb[:, :, third : 2 * third], cols_t[:, :, third : 2 * third]
            )
            nc.gpsimd.dma_start(
                colsb[:, :, 2 * third : LA], cols_t[:, :, 2 * third : LA]
            )
            mid = (LA + Lc) // 2
            nc.sync.dma_start(colsb[:, :, LA:mid], cols_t[:, :, LA:mid])
            nc.scalar.dma_start(colsb[:, :, mid:Lc], cols_t[:, :, mid:Lc])
            ct = colsb[:].tensor

            # ---- box decode (original order), all cores redundantly ----
            dr = wp.tile([128, 64, 4], f32, tag="dr")
            nc.vector.tensor_scalar_max(dr[:], dl[:], 0.0)
            bx = wp.tile([128, 64, 4], f32, tag="bx")
            for c, (sgn, lc) in enumerate([(-1, 0), (-1, 1), (1, 0), (1, 1)]):
                nc.vector.scalar_tensor_tensor(
                    bx[:, :, c], dr[:, :, c], sgn * stride_f, ll[:, :, lc],
                    op0=Alu.mult, op1=Alu.add,
                )
            nc.sync.dma_start(boxes_o.rearrange("(p f) c -> p f c", p=128), bx[:])

            def cwin1(q, t):  # single chunk t (global in 0..CPC): [128, W]
                return AP(ct, q * Lc + t * CH, [[4 * Lc, 128], [1, W]])

            def rcol(q, t):  # [128,1] scalar AP for chunk t
                return rowsb[:, t * 5 + q : t * 5 + q + 1]

            for h in range(NHALF):
                # asum on TensorE: psum[p, f] = area_r[p]*1 + 1*area_c[f]
                asum = pp.tile([128, HPC, W], f32, tag="asum")
                for tl in range(HPC):
                    t = h * HPC + tl
                    nc.tensor.matmul(
                        asum[:, tl, :],
                        mmlb[:, t * CH : (t + 1) * CH],
                        mmrb[:, t * CH : t * CH + W],
                        start=True, stop=True,
                    )

                # fused overlap extents: one custom-DVE pass per chunk+axis
                wr = wp.tile([128, HPC, W], f32, tag="wr")
                hr = wp.tile([128, HPC, W], f32, tag="hr")
                for tl in range(HPC):
                    t = h * HPC + tl
                    nc.vector._custom_dve(
                        edge_op, out=wr[:, tl, :],
                        in0=cwin1(2, t), in1=cwin1(0, t),
                        s0=rcol(2, t), s1=rcol(0, t),
                    )
                    nc.vector._custom_dve(
                        edge_op, out=hr[:, tl, :],
                        in0=cwin1(3, t), in1=cwin1(1, t),
                        s0=rcol(3, t), s1=rcol(1, t),
                    )

                inter = wp.tile([128, HPC, W], f32, tag="inter")
                nc.vector.tensor_tensor(inter[:], wr[:], hr[:], op=Alu.mult)

                # pred = (inter*3) > asum   (safe: >=150 ulp data margin)
                pr = wp.tile([128, HPC, W], u8, tag="pr")
                nc.vector.scalar_tensor_tensor(
                    pr[:], inter[:], 3.0, asum[:], op0=Alu.mult, op1=Alu.is_gt
                )
                # one DMA per quarter; dram AP reordered to match [128, HPC, W]
                (nc.sync if h % 2 == 0 else nc.scalar).dma_start(
                    AP(pred_o.tensor, h * HPC * 128 * W,
                       [[W, 128], [128 * W, HPC], [1, W]]),
                    pr[:],
                )

    nc.compile()
    return nc


def kernel(deltas, locations, scores, class_ids, stride):
    deltas = np.asarray(deltas, np.float32)
    locations = np.asarray(locations, np.float32)
    scores = np.asarray(scores, np.float32)
    class_ids = np.asarray(class_ids, np.int32)
    stride_f = float(np.asarray(stride))
    n = deltas.shape[0]
    assert n == N

    # ---- host: decode (for sort/band prep only; boxes output comes from
    # the device), class offsets, x1-sort, window size ----
    dd = np.clip(deltas, 0, None)
    xc, yc = locations[:, 0], locations[:, 1]
    s8 = np.float32(stride_f)
    bx = np.stack(
        [xc - dd[:, 0] * s8, yc - dd[:, 1] * s8,
         xc + dd[:, 2] * s8, yc + dd[:, 3] * s8], axis=1
    ).astype(np.float32)
    mc = bx.max()
    off = (class_ids.astype(np.float32) * (mc + np.float32(1.0))).astype(np.float32)
    b = (bx + off[:, None]).astype(np.float32)
    areas = ((b[:, 2] - b[:, 0]) * (b[:, 3] - b[:, 1])).astype(np.float32)

    xorder = np.argsort(b[:, 0], kind="stable")
    bsx = b[xorder]
    areax = areas[xorder]
    x1s, y1s, x2s, y2s = bsx[:, 0], bsx[:, 1], bsx[:, 2], bsx[:, 3]

    # exact candidate window: for row i all j>i with x1[j] <= x2[i]
    ends = np.searchsorted(x1s, x2s, side="right")
    wneed = max(
        int(ends[t * CH : (t + 1) * CH].max()) - t * CH for t in range(NCHUNK)
    )
    W = max(256, int(np.ceil(wneed / 128.0)) * 128)

    key = (W, stride_f)
    if key not in _PROGRAM_CACHE:
        _PROGRAM_CACHE[key] = _build_program(W, stride_f)
    nc = _PROGRAM_CACHE[key]

    # ---- padded column arrays (pad boxes can never overlap: x1=+huge) ----
    Lc = CPC * CH + W
    PAD = np.float32(3e38)
    L = n + W
    colq = np.empty((5, L), np.float32)
    colq[0, :n] = x1s; colq[0, n:] = PAD
    colq[1, :n] = y1s; colq[1, n:] = PAD
    colq[2, :n] = x2s; colq[2, n:] = PAD
    colq[3, :n] = y2s; colq[3, n:] = PAD
    colq[4, :n] = areax; colq[4, n:] = 0.0

    rowq = np.stack([x1s, y1s, x2s, y2s, areax], axis=1)  # [n, 5]

    in_maps = []
    for c in range(NCORES):
        s0 = c * CPC * CH
        cols_c = np.ascontiguousarray(
            np.broadcast_to(colq[None, :4, s0 : s0 + Lc], (128, 4, Lc))
        )
        rows_c = np.ascontiguousarray(
            rowq[s0 : s0 + CPC * CH].reshape(CPC, CH, 5).transpose(1, 0, 2)
        ).reshape(128, CPC * 5)
        mml_c = np.ones((2, CPC * CH), np.float32)
        mml_c[0] = areax[s0 : s0 + CPC * CH]
        mmr_c = np.ones((2, Lc), np.float32)
        mmr_c[1] = colq[4, s0 : s0 + Lc]
        in_maps.append(
            {
                "deltas": deltas,
                "locations": locations,
                "cols": cols_c,
                "rows": rows_c,
                "mml": mml_c,
                "mmr": mmr_c,
            }
        )

    from concourse import bass_utils

    res = bass_utils.run_bass_kernel_spmd(
        nc, in_maps, core_ids=list(range(NCORES))
    )
    results = res.results

    boxes_out = results[0]["boxes"]

    predb = np.concatenate(
        [results[c]["pred"] for c in range(NCORES)], axis=0
    )  # [NCHUNK, 128, W]

    # ---- host: edge extraction + greedy sweep ----
    tt, pp, ff = np.nonzero(predb)
    i = tt * CH + pp
    j = tt * CH + ff
    m = (j > i) & (j < n)
    i, j = i[m], j[m]

    sorder = np.argsort(-scores, kind="stable")
    srank = np.empty(n, np.int64)
    srank[sorder] = np.arange(n)
    si = srank[xorder[i]]
    sj = srank[xorder[j]]
    lo = np.minimum(si, sj)
    hi = np.maximum(si, sj)

    keep_s = np.ones(n, bool)
    if len(lo):
        perm = np.argsort(lo, kind="stable")
        lo, hi = lo[perm], hi[perm]
        uniq, start = np.unique(lo, return_index=True)
        start = np.append(start, len(lo))
        for k in range(len(uniq)):
            if keep_s[uniq[k]]:
                keep_s[hi[start[k] : start[k + 1]]] = False
    keep_mask = np.zeros(n, bool)
    keep_mask[sorder] = keep_s

    return boxes_out, keep_mask
